# revision 19
# baseline (speedup 1.0000x reference)
"""Multi-head causal attention (B=2, S=4096, D=512, H=8, DK=64) on 8 TRN2
NeuronCores.

Sharding: batch x head-pair for attention (core c: batch c//4, heads
{2*(c%4), 2*(c%4)+1} end-to-end), then 8-way-interleaved output rows for
the projection: head mixing is a single 8-core AllToAll per sequence
piece, after which core d applies Wo to its 8 owned 128-row blocks —
rows 512*(d//4 + 2k) + 128*(d%4), k=0..3, of BOTH batches.

Per-core dataflow (everything "T" is d-major, i.e. feature dim on SBUF
partitions, which is what the PE matmul contraction needs). The whole
datapath is bf16 (casting DMAs on load) with fp32 PSUM accumulation —
rel err ~4e-3 vs the 2e-2 gate; bf16 PE transposes run 2x the fp32 rate
and their PSUM evacuations get the 2x DVE mode:
  QT/KT/VT via PE transpose (bf16) -> qT/kT = W^T @ XT, v = V @ Wv
  sT[t, sq] = k^T q (transposed scores; K=64 contraction, bf16)
  wT = exp(sT/8) via ScalarE straight out of PSUM, over causally-live
       columns only; the causal mask is a post-exp 0/1 multiply on the
       128x128 diagonal blocks, off the scores->exp chain (zeroed
       weights drop out of the ones-column denominator below);
       strictly-above-diagonal blocks are never computed
  oT_aug = [v | 1]^T @ wT accumulated over t-chunks in one PSUM bank;
       row 64 is the softmax denominator (no separate reduction pass)
  oT = oT_aug[:64] * (1/oT_aug[64]) broadcast via GPSIMD partition_broadcast
  oT (bf16) staged per chunk to a DRAM bounce in destination-major
       stripe order and exchanged by an 8-way AllToAll in two pieces
       (q-chunks 0-5 / 6-7), each fired the moment its last chunk
       normalizes. AllToAll, not AllGather: each core only needs the
       column stripes of the other heads covering ITS output rows, so
       the collective moves 1/4 the bytes (cost = ~15us fixed +
       bytes/40GBps in the cost model; 4-core AllToAll is unsupported,
       hence one 8-way exchange carrying both batches). All APs are
       rank-independent: stripes are routed by destination block, so no
       partition_id-conditional DMAs or dynamic slices exist anywhere.
  y = oT_all^T @ Wo (bf16), one [128, 512] block per owned row-block,
       stored fp32; _assemble interleaves the blocks back.

Phase 3 (consume DMAs + out-proj) is emitted under tc.tile_wait_until
scheduling hints that pin it after the attention stream in every
in-order engine queue: the Tile scheduler's collective cost model is
optimistic, and letting it hoist out-proj matmuls between attention
instructions makes the real PE queue stall on the exchange semaphore
(measured +28us). Consume DMAs also must NOT be emitted mid-loop on the
sync queue, or later bounce-in DMAs queue behind their collective wait
and delay the next AllToAll launch.

Engine budget (cost model, per core): PE ~165us busy (scores+oT ~109us,
transposes ~20us, projections ~24us, out-proj ~7us), ScalarE ~143us
(144 exp instructions — exp exists on no other engine), VectorE ~92us,
collectives 35/22us serialized on the collective engine (two pieces,
not three: a middle piece only adds a serialized fixed cost). PSUM (8
banks): 2 alternating single-buffer score pools + 2 oT accumulators +
2-buffer mm scratch — the full budget. TimelineSim: ~234us end-to-end
(268 for the AllGather baseline); the last exchange launches at ~200us
(attention end, ScalarE-paced) and only a ~12us tail follows it.
Measured dead ends on HW (~+3-6us each): DMA-xbar transposes instead of
PE, normalizing straight out of PSUM (even for just the final chunk),
4-way AllGather splits, software-pipelining the attention inner loop.
"""

import sys

sys.path.insert(0, "/opt/trn_rl_repo")

import numpy as np

import concourse.bass as bass
import concourse.mybir as mybir
import concourse.tile as tile
from concourse import bacc
from concourse.bass import ds, ts
from concourse.bass_utils import run_bass_kernel_spmd
from concourse.masks import make_identity

B, S, D, H, DK = 2, 4096, 512, 8, 64
SQ, TC = 512, 128  # q-chunk rows, t-chunk rows
NSL = S // SQ  # 8 row slices
NCHUNK = S // TC  # 32 t-chunks
f32 = mybir.dt.float32
f32r = mybir.dt.float32r
bf16 = mybir.dt.bfloat16
AF = mybir.ActivationFunctionType
ALU = mybir.AluOpType

_CACHED_NC = None


def attention_chunk(nc, pool, sA, sB, otp, mask128, qt_sl, kt_sl, v_sl, ot_half,
                    bounce_in_part, crel, gi0):
    """Attention for q-chunk c, both heads, t-chunks 0..4(c+1)-1.

    Each scores/exp group holds one t-chunk for BOTH heads ([128, 2, 512]);
    oT accumulates per head in its own PSUM bank across the t loop."""
    c = len(qt_sl) - 1  # current q-chunk == latest slice
    n_tc = 4 * (c + 1)
    ot_ps = [None, None]
    for tcg in range(n_tc):
        r = tcg - 4 * c
        sl, lc = tcg // 4, tcg % 4
        n0 = 128 * r if r >= 0 else 0
        gi = gi0 + tcg
        sp = (sA if gi % 2 == 0 else sB).tile(
            [128, 2, 512], f32, tag="sA" if gi % 2 == 0 else "sB"
        )
        for h in range(2):
            nc.tensor.matmul(
                sp[:, h, n0:512],
                lhsT=kt_sl[sl][64 * h : 64 * h + 64, ts(lc, 128)],
                rhs=qt_sl[c][64 * h : 64 * h + 64, n0:512],
                start=True,
                stop=True,
            )
        wt = pool("wt", 6).tile([128, 2, 512], bf16, tag="wt")
        # diagonal groups: exp only the causally-live columns (cols < n0 are
        # stale PSUM never read by the oT matmuls below)
        nc.scalar.activation(wt[:, :, n0:512], sp[:, :, n0:512], AF.Exp, scale=0.125)
        if r >= 0:
            # causal mask as a post-exp 0/1 multiply on the diagonal block:
            # cheaper (bf16 SBUF 2x DVE mode vs fp32 PSUM add) and off the
            # scores->exp critical chain; the softmax denominator comes from
            # the ones-column oT sum below, so zeroed weights drop out of it
            mask2 = bass.AP(
                tensor=mask128.tensor,
                offset=mask128.offset,
                ap=[mask128.ap[0], [0, 2], [1, 128]],
            )
            nc.vector.tensor_mul(
                wt[:, :, n0 : n0 + 128], wt[:, :, n0 : n0 + 128], mask2
            )
        for h in range(2):
            if tcg == 0:
                ot_ps[h] = otp.tile([128, 512], f32, tag="otp", name=f"otp_c{c}h{h}")
            nc.tensor.matmul(
                ot_ps[h][0:65, n0:512],
                lhsT=v_sl[sl][:, lc, 65 * h : 65 * h + 65],
                rhs=wt[:, h, n0:512],
                start=(tcg == 0),
                stop=(tcg == n_tc - 1),
            )
    for h in range(2):
        # one cheap copy releases the PSUM accumulator immediately; the
        # normalize chain then runs off the oT-accumulation critical path
        # (normalizing straight out of PSUM measured 3-6us slower on HW)
        ot_raw = pool("otraw", 4).tile([65, 512], f32, tag="otraw",
                                       name=f"otraw_c{c}h{h}")
        nc.vector.tensor_copy(ot_raw, ot_ps[h][0:65, :])
        recip = pool("recip", 2).tile([1, 512], f32, tag="recip")
        nc.vector.reciprocal(recip, ot_raw[64:65, :])
        rbc = pool("rbc", 2).tile([64, 512], f32, tag="rbc")
        nc.gpsimd.partition_broadcast(rbc, recip)
        nc.vector.tensor_mul(
            ot_half[64 * h : 64 * h + 64, crel, :], ot_raw[0:64, :], rbc
        )
    # stage to the DRAM bounce in destination-major order: chunk c's four
    # 128-col stripes go to A2A ranks 4*(c%2)+s for s in 0..3 — rank d owns
    # output rows (stripe d%4, chunks == d//4 mod 2) of BOTH batches
    q = c % 2
    lj = crel // 2
    nc.sync.dma_start(
        bounce_in_part.rearrange("(d p) (j c) -> p d j c", p=128, c=128)[
            :, 4 * q : 4 * q + 4, lj, :
        ],
        ot_half.rearrange("p j (s c) -> p s j c", c=128)[:, :, crel, :],
    )


def _build_body(nc, tc, Q, K, V, Wq, Wk, Wv, Wo, Y):
    ctx_pools = {}

    def pool(name, bufs, space="SBUF"):
        if name not in ctx_pools:
            ctx_pools[name] = tc.alloc_tile_pool(name=name, bufs=bufs, space=space)
        return ctx_pools[name]

    def psum_pool(name, bufs):
        return pool(name, bufs, space="PSUM")

    const = pool("const", 1)
    ident16 = const.tile([128, 128], bf16, tag="id16")
    make_identity(nc, ident16)
    # multiplicative causal mask for a 128x128 diagonal block: keep (1)
    # where col >= row, else 0 — applied to the exp'd weights
    mask128 = const.tile([128, 128], bf16, tag="mask")
    nc.vector.memset(mask128, 1.0)
    nc.gpsimd.affine_select(
        out=mask128,
        in_=mask128,
        compare_op=ALU.is_ge,
        fill=0.0,
        base=0,
        channel_multiplier=-1,
        pattern=[[1, 128]],
    )


    mm = psum_pool("mm", 2)  # [128, 512] single-bank tiles, double-buffered
    sA = psum_pool("sA", 1)  # [128, 2, 512] scores group (even)
    sB = psum_pool("sB", 1)  # [128, 2, 512] scores group (odd)
    otp = psum_pool("otp", 2)  # [128, 512] oT accumulator

    qt_sl, kt_sl, v_sl = [], [], []
    GI = [0]

    # attention staging: heads paired per scores group (PE row-group
    # concurrency); oT written bf16, exchanged in three overlapped pieces
    # (chunks 0-3 / 4-5 / 6-8) so only the last small piece sits on the tail.
    AG_SPLIT = [(0, 6), (6, 8)]  # [c0, c1) chunk ranges
    # Head mixing is one 8-way AllToAll per piece, not per-batch AllGathers:
    # each core only needs the column stripes of the other heads that cover
    # ITS output rows, so routing just those moves 1/4 the bytes (collective
    # time is fixed-cost + bytes/bw; 4-core AllToAll is unsupported, hence
    # 8-way with output rows from BOTH batches per core). A2A rank d owns,
    # in each batch, the 128-row blocks 512*j + 128*(d%4) for chunks
    # j == d//4 (mod 2) — every piece carries data for every core, the
    # out-proj streams behind each piece, and all APs are rank-independent.
    ot_all = [
        pool(f"otall{i}", 1).tile(
            [128, 8, (c1 - c0) // 2, 128], bf16, tag=f"otall{i}", name=f"otall{i}"
        )
        for i, (c0, c1) in enumerate(AG_SPLIT)
    ]
    ot_parts = [
        pool("ot", 1).tile([128, c1 - c0, 512], bf16, tag=f"otp{i}", name=f"otsb{i}")
        for i, (c0, c1) in enumerate(AG_SPLIT)
    ]
    dram = pool("dram", 1, space="DRAM")
    bounce_in = [
        dram.tile([1024, (c1 - c0) * 64], bf16, tag=f"bin{i}", name=f"bin{i}")
        for i, (c0, c1) in enumerate(AG_SPLIT)
    ]
    bounce_out = [
        dram.tile([1024, (c1 - c0) * 64], bf16, tag=f"bout{i}", name=f"bout{i}")
        for i, (c0, c1) in enumerate(AG_SPLIT)
    ]

    # ---------------- phase 1: load, transpose, project ----------------
    Qr = Q.rearrange("(s g p) d -> s p g d", p=128, g=4)
    Kr = K.rearrange("(s g p) d -> s p g d", p=128, g=4)
    Vr = V.rearrange("(s g p) d -> s p g d", p=128, g=4)

    wq_sb = wk_sb = wv_sb = wo_sb = None
    for s in range(NSL):
        # --- Q and K: bf16 path (casting DMAs; bf16 transposes run 2x the
        # fp32 PE rate and their PSUM evacuations get the 2x DVE mode; fp32
        # PSUM accumulation in every matmul keeps rel err ~4e-3) ---
        xq = pool("xin", 4).tile([128, 4, 512], bf16, tag="xin")
        nc.gpsimd.dma_start(xq, Qr[s])  # casting DMA f32 -> bf16
        xk = pool("xin", 4).tile([128, 4, 512], bf16, tag="xin")
        nc.gpsimd.dma_start(xk, Kr[s])
        if s == 0:
            # weight loads queued behind the first slice so they don't
            # delay the first transposes
            wq_sb = const.tile([128, 4, 128], bf16, tag="wq")
            nc.gpsimd.dma_start(wq_sb, Wq.rearrange("(c p) k -> p c k", p=128))
            wk_sb = const.tile([128, 4, 128], bf16, tag="wk")
            nc.gpsimd.dma_start(wk_sb, Wk.rearrange("(c p) k -> p c k", p=128))
            wv_sb = const.tile([128, 4, 128], bf16, tag="wv")
            nc.gpsimd.dma_start(wv_sb, Wv.rearrange("(c p) k -> p c k", p=128))
            wo_sb = const.tile([128, 4, 512], bf16, tag="wo")
            nc.gpsimd.dma_start(wo_sb, Wo.rearrange("(c p) n -> p c n", p=128))
        xtq = pool("xt", 3).tile([128, 4, 512], bf16, tag="xtqk")
        xtk = pool("xt", 3).tile([128, 4, 512], bf16, tag="xtqk")
        for x_sl, xt_sb in ((xq, xtq), (xk, xtk)):
            for dc in range(4):
                t_ps = mm.tile([128, 512], bf16, tag="mm", name=f"tps_{s}_{dc}")
                for g in range(4):
                    nc.tensor.transpose(
                        t_ps[:, ts(g, 128)], x_sl[:, g, ts(dc, 128)], ident16
                    )
                nc.vector.tensor_copy(xt_sb[:, dc, :], t_ps)
        # qT/kT projections (both heads of the pair): [128, 512]
        qt_ps = mm.tile([128, 512], f32, tag="mm")
        for dc in range(4):
            nc.tensor.matmul(
                qt_ps,
                lhsT=wq_sb[:, dc, :],
                rhs=xtq[:, dc, :],
                start=(dc == 0),
                stop=(dc == 3),
            )
        qt = pool("qt", NSL).tile([128, 512], bf16, tag="qt")
        nc.vector.tensor_copy(qt, qt_ps)
        qt_sl.append(qt)
        kt_ps = mm.tile([128, 512], f32, tag="mm")
        for dc in range(4):
            nc.tensor.matmul(
                kt_ps,
                lhsT=wk_sb[:, dc, :],
                rhs=xtk[:, dc, :],
                start=(dc == 0),
                stop=(dc == 3),
            )
        kt = pool("kt", NSL).tile([128, 512], bf16, tag="kt")
        nc.vector.tensor_copy(kt, kt_ps)
        kt_sl.append(kt)

        # --- V: bf16 path ---
        xv = pool("xinv", 2).tile([128, 4, 512], bf16, tag="xinv")
        nc.gpsimd.dma_start(xv, Vr[s])  # casting DMA f32 -> bf16
        xtv = pool("xtv", 3).tile([128, 4, 512], bf16, tag="xtv")
        for dc in range(4):
            t_ps = mm.tile([128, 512], bf16, tag="mm", name=f"tpsv_{s}_{dc}")
            for g in range(4):
                nc.tensor.transpose(
                    t_ps[:, ts(g, 128)], xv[:, g, ts(dc, 128)], ident16
                )
            nc.vector.tensor_copy(xtv[:, dc, :], t_ps)
        # v projection, t-major: per t-chunk [128, 2*64]; interleave into
        # v_aug [128, 4, 130] with a ones column per head at 65h+64
        vp = mm.tile([128, 512], f32, tag="mm")
        for tcl in range(4):
            for dc in range(4):
                nc.tensor.matmul(
                    vp[:, ts(tcl, 128)],
                    lhsT=xtv[:, dc, ts(tcl, 128)],
                    rhs=wv_sb[:, dc, :],
                    start=(dc == 0),
                    stop=(dc == 3),
                )
        va = pool("v", NSL).tile([128, 4, 130], bf16, tag="v")
        nc.vector.memset(va.rearrange("p c (h k) -> p c h k", k=65)[:, :, :, 64:65], 1.0)
        nc.vector.tensor_copy(
            va.rearrange("p c (h k) -> p c h k", k=65)[:, :, :, 0:64],
            vp.rearrange("p (c h k) -> p c h k", c=4, h=2),
        )
        v_sl.append(va)

        part = next(i for i, (c0, c1) in enumerate(AG_SPLIT) if c0 <= s < c1)
        attention_chunk(nc, pool, sA, sB, otp, mask128, qt_sl, kt_sl, v_sl,
                        ot_parts[part], bounce_in[part],
                        s - AG_SPLIT[part][0], GI[0])
        GI[0] += 4 * (s + 1)
        if s == AG_SPLIT[part][1] - 1:
            nc.gpsimd.collective_compute(
                "AllToAll",
                ALU.bypass,
                replica_groups=[[0, 1, 2, 3, 4, 5, 6, 7]],
                ins=[bounce_in[part].opt()],
                outs=[bounce_out[part].opt()],
            )

    # ------------- phase 3: consume gathers + output projection ---------
    # All of phase 3 is emitted under a scheduling wait-hint that places it
    # after the attention stream in each in-order engine queue: the Tile
    # scheduler's collective-cost model is optimistic, and letting it hoist
    # out-proj matmuls between attention instructions makes the real PE
    # queue stall on the AllGather semaphore (measured +28us in sim). PE
    # idles for the whole tail AllGather anyway — plenty for all 8 blocks.
    # Consume DMAs sit here, not in the loop, so mid-loop bounce-in DMAs on
    # the sync queue are never stuck behind a consume's collective wait.
    def consume(part):
        # the AllToAll output is source-major: rows [128*s, 128*s+128) hold
        # core s's 128 head-features over my column stripes, contiguous —
        # sources 0-3 are batch-0 heads, 4-7 batch-1 (the tail piece is
        # spread across the three DMA-capable queues, all idle by then)
        engs = (
            (nc.sync,) * 8
            if part < len(AG_SPLIT) - 1
            else (nc.sync, nc.scalar, nc.gpsimd) * 3
        )
        for s in range(8):
            engs[s].dma_start(ot_all[part][:, s], bounce_out[part][ts(s, 128), :])

    def out_proj(bb, k):
        # my k-th owned block of batch bb: global rows 512*(par+2k) +
        # 128*(rank%4), with par = rank//4 — piece and local-chunk indices
        # are rank-independent (k<3 -> piece 0 local k; k=3 -> piece 1
        # local 0); batch bb's head-features are A2A sources 4*bb+dc
        part, j = (0, k) if k < 3 else (1, 0)
        st = 4 * bb + k
        ym = mm.tile([128, 512], f32, tag="mm", name=f"ym_{st}")
        for dc in range(4):
            nc.tensor.matmul(
                ym,
                lhsT=ot_all[part][:, 4 * bb + dc, j, :],
                rhs=wo_sb[:, dc, :],
                start=(dc == 0),
                stop=(dc == 3),
            )
        y_sb = pool("y", 4).tile([128, 512], f32, tag="y")
        if st % 2 == 0:
            nc.scalar.copy(y_sb, ym)
            nc.sync.dma_start(Y[ts(st, 128), :], y_sb)
        else:
            nc.vector.tensor_copy(y_sb, ym)
            nc.scalar.dma_start(Y[ts(st, 128), :], y_sb)

    with tc.tile_wait_until(0.20):
        consume(0)
        for k in range(3):
            for bb in range(2):
                out_proj(bb, k)
    with tc.tile_wait_until(0.24):
        consume(1)
        for bb in range(2):
            out_proj(bb, 3)

    for p in reversed(list(ctx_pools.values())):
        p.release()


def _build(loop=1):
    global _CACHED_NC
    if loop == 1 and _CACHED_NC is not None:
        return _CACHED_NC
    nc = bacc.Bacc("TRN2", num_devices=8)
    Q = nc.dram_tensor("Q", [S, D], f32, kind="ExternalInput")
    K = nc.dram_tensor("K", [S, D], f32, kind="ExternalInput")
    V = nc.dram_tensor("V", [S, D], f32, kind="ExternalInput")
    Wq = nc.dram_tensor("Wq", [D, 128], f32, kind="ExternalInput")
    Wk = nc.dram_tensor("Wk", [D, 128], f32, kind="ExternalInput")
    Wv = nc.dram_tensor("Wv", [D, 128], f32, kind="ExternalInput")
    Wo = nc.dram_tensor("Wo", [D, D], f32, kind="ExternalInput")
    Y = nc.dram_tensor("Y", [1024, D], f32, kind="ExternalOutput")
    with tile.TileContext(nc) as tcx:
        for _ in range(loop):
            _build_body(nc, tcx, Q, K, V, Wq, Wk, Wv, Wo, Y)
    nc.finalize()
    if loop == 1:
        _CACHED_NC = nc
    return nc


def _in_maps(inputs):
    Q, K, V = (np.asarray(inputs[k], np.float32) for k in ("Q", "K", "V"))
    Wq, Wk, Wv, Wo = (
        np.asarray(inputs[k], np.float32) for k in ("Wq", "Wk", "Wv", "Wo")
    )
    in_maps = []
    for c in range(8):
        b, hp = c // 4, c % 4
        in_maps.append(
            {
                "Q": np.ascontiguousarray(Q[b]),
                "K": np.ascontiguousarray(K[b]),
                "V": np.ascontiguousarray(V[b]),
                "Wq": np.ascontiguousarray(
                    np.concatenate([Wq[2 * hp], Wq[2 * hp + 1]], axis=1)
                ),
                "Wk": np.ascontiguousarray(
                    np.concatenate([Wk[2 * hp], Wk[2 * hp + 1]], axis=1)
                ),
                "Wv": np.ascontiguousarray(
                    np.concatenate([Wv[2 * hp], Wv[2 * hp + 1]], axis=1)
                ),
                "Wo": Wo,
            }
        )
    return in_maps


def _assemble(per_core_results):
    # A2A rank d returns Y blocks st = 4*bb + k: batch bb's global rows
    # 512*(d//4 + 2*k) + 128*(d%4) .. +128
    out = np.empty((B, S, D), np.float32)
    for d in range(8):
        par, sp = d // 4, d % 4
        y = per_core_results[d]["Y"].reshape(2, 4, 128, D)
        for bb in range(2):
            for k in range(4):
                r0 = 512 * (par + 2 * k) + 128 * sp
                out[bb, r0 : r0 + 128] = y[bb, k]
    return out


def kernel(Q, K, V, Wq, Wk, Wv, Wo):
    nc = _build()
    in_maps = _in_maps(
        {"Q": Q, "K": K, "V": V, "Wq": Wq, "Wk": Wk, "Wv": Wv, "Wo": Wo}
    )
    res = run_bass_kernel_spmd(nc, in_maps, core_ids=list(range(8)))
    return _assemble(res.results)



# revision 21
# speedup vs baseline: 1.0282x; 1.0282x over previous
"""Multi-head causal attention (B=2, S=4096, D=512, H=8, DK=64) on 8 TRN2
NeuronCores.

Sharding: batch x head-pair for attention (core c: batch c//4, heads
{2*(c%4), 2*(c%4)+1} end-to-end), then 8-way-interleaved output rows for
the projection: head mixing is a single 8-core AllToAll per sequence
piece, after which core d applies Wo to its 8 owned 128-row blocks —
rows 512*(d//4 + 2k) + 128*(d%4), k=0..3, of BOTH batches.

Per-core dataflow (everything "T" is d-major, i.e. feature dim on SBUF
partitions, which is what the PE matmul contraction needs). The whole
datapath is bf16 (casting DMAs on load) with fp32 PSUM accumulation —
rel err ~4e-3 vs the 2e-2 gate; bf16 PE transposes run 2x the fp32 rate
and their PSUM evacuations get the 2x DVE mode:
  QT/KT/VT via PE transpose (bf16) -> qT/kT = W^T @ XT, v = V @ Wv
  sT[t, sq] = k^T q (transposed scores; K=64 contraction, bf16)
  wT = exp(sT/8) via ScalarE straight out of PSUM, over causally-live
       columns only; the causal mask is a post-exp 0/1 multiply on the
       128x128 diagonal blocks, off the scores->exp chain (zeroed
       weights drop out of the ones-column denominator below);
       strictly-above-diagonal blocks are never computed
  oT_aug = [v | 1]^T @ wT accumulated over t-chunks in one PSUM bank;
       row 64 is the softmax denominator (no separate reduction pass)
  oT = oT_aug[:64] * (1/oT_aug[64]) broadcast via GPSIMD partition_broadcast
  oT (bf16) staged per chunk to a DRAM bounce in destination-major
       stripe order and exchanged by an 8-way AllToAll in three pieces
       (q-chunks 0-3 / 4-5 / 6-7), each fired the moment its last chunk
       normalizes. AllToAll, not AllGather: each core only needs the
       column stripes of the other heads covering ITS output rows, so
       the collective moves 1/4 the bytes (cost = ~15us fixed +
       bytes/40GBps in the cost model; 4-core AllToAll is unsupported,
       hence one 8-way exchange carrying both batches). All APs are
       rank-independent: stripes are routed by destination block, so no
       partition_id-conditional DMAs or dynamic slices exist anywhere.
  y = oT_all^T @ Wo (bf16), one [128, 512] block per owned row-block,
       stored fp32; _assemble interleaves the blocks back.

Phase 3 (consume DMAs + out-proj) is emitted under tc.tile_wait_until
scheduling hints that pin it after the attention stream in every
in-order engine queue: the Tile scheduler's collective cost model is
optimistic, and letting it hoist out-proj matmuls between attention
instructions makes the real PE queue stall on the exchange semaphore
(measured +28us). Consume DMAs also must NOT be emitted mid-loop on the
sync queue, or later bounce-in DMAs queue behind their collective wait
and delay the next AllToAll launch.

Engine budget (cost model, per core): PE ~165us busy (scores+oT ~109us,
transposes ~20us, projections ~24us, out-proj ~7us), ScalarE ~143us
(144 exp instructions — exp exists on no other engine), VectorE ~92us,
collectives 28/22/22us serialized on the collective engine. PSUM (8
banks): 2 alternating single-buffer score pools + 2 oT accumulators +
2-buffer mm scratch — the full budget. TimelineSim: ~234us end-to-end
(268 for the AllGather baseline); the last exchange launches at ~200us
(attention end, ScalarE-paced) and only a ~12us tail follows it.
Measured dead ends on HW (~+3-7us each): DMA-xbar transposes instead of
PE, normalizing straight out of PSUM (even for just the final chunk),
4-way AllGather splits, software-pipelining the attention inner loop,
and merging the first two A2A pieces into one (0,6) piece (sim-neutral
but 291.9us vs 285.1us on HW — HW collective cost is NOT dominated by a
per-instance fixed floor; keep the early exchanges small and early).
This 3-piece A2A version measured 285071ns on HW, rel err 4.16e-3.
"""

import sys

sys.path.insert(0, "/opt/trn_rl_repo")

import numpy as np

import concourse.bass as bass
import concourse.mybir as mybir
import concourse.tile as tile
from concourse import bacc
from concourse.bass import ds, ts
from concourse.bass_utils import run_bass_kernel_spmd
from concourse.masks import make_identity

B, S, D, H, DK = 2, 4096, 512, 8, 64
SQ, TC = 512, 128  # q-chunk rows, t-chunk rows
NSL = S // SQ  # 8 row slices
NCHUNK = S // TC  # 32 t-chunks
f32 = mybir.dt.float32
f32r = mybir.dt.float32r
bf16 = mybir.dt.bfloat16
AF = mybir.ActivationFunctionType
ALU = mybir.AluOpType

_CACHED_NC = None


def attention_chunk(nc, pool, sA, sB, otp, mask128, qt_sl, kt_sl, v_sl, ot_half,
                    bounce_in_part, crel, gi0):
    """Attention for q-chunk c, both heads, t-chunks 0..4(c+1)-1.

    Each scores/exp group holds one t-chunk for BOTH heads ([128, 2, 512]);
    oT accumulates per head in its own PSUM bank across the t loop."""
    c = len(qt_sl) - 1  # current q-chunk == latest slice
    n_tc = 4 * (c + 1)
    ot_ps = [None, None]
    for tcg in range(n_tc):
        r = tcg - 4 * c
        sl, lc = tcg // 4, tcg % 4
        n0 = 128 * r if r >= 0 else 0
        gi = gi0 + tcg
        sp = (sA if gi % 2 == 0 else sB).tile(
            [128, 2, 512], f32, tag="sA" if gi % 2 == 0 else "sB"
        )
        for h in range(2):
            nc.tensor.matmul(
                sp[:, h, n0:512],
                lhsT=kt_sl[sl][64 * h : 64 * h + 64, ts(lc, 128)],
                rhs=qt_sl[c][64 * h : 64 * h + 64, n0:512],
                start=True,
                stop=True,
            )
        wt = pool("wt", 6).tile([128, 2, 512], bf16, tag="wt")
        # diagonal groups: exp only the causally-live columns (cols < n0 are
        # stale PSUM never read by the oT matmuls below)
        nc.scalar.activation(wt[:, :, n0:512], sp[:, :, n0:512], AF.Exp, scale=0.125)
        if r >= 0:
            # causal mask as a post-exp 0/1 multiply on the diagonal block:
            # cheaper (bf16 SBUF 2x DVE mode vs fp32 PSUM add) and off the
            # scores->exp critical chain; the softmax denominator comes from
            # the ones-column oT sum below, so zeroed weights drop out of it
            mask2 = bass.AP(
                tensor=mask128.tensor,
                offset=mask128.offset,
                ap=[mask128.ap[0], [0, 2], [1, 128]],
            )
            nc.vector.tensor_mul(
                wt[:, :, n0 : n0 + 128], wt[:, :, n0 : n0 + 128], mask2
            )
        for h in range(2):
            if tcg == 0:
                ot_ps[h] = otp.tile([128, 512], f32, tag="otp", name=f"otp_c{c}h{h}")
            nc.tensor.matmul(
                ot_ps[h][0:65, n0:512],
                lhsT=v_sl[sl][:, lc, 65 * h : 65 * h + 65],
                rhs=wt[:, h, n0:512],
                start=(tcg == 0),
                stop=(tcg == n_tc - 1),
            )
    for h in range(2):
        # one cheap copy releases the PSUM accumulator immediately; the
        # normalize chain then runs off the oT-accumulation critical path
        # (normalizing straight out of PSUM measured 3-6us slower on HW)
        ot_raw = pool("otraw", 4).tile([65, 512], f32, tag="otraw",
                                       name=f"otraw_c{c}h{h}")
        nc.vector.tensor_copy(ot_raw, ot_ps[h][0:65, :])
        recip = pool("recip", 2).tile([1, 512], f32, tag="recip")
        nc.vector.reciprocal(recip, ot_raw[64:65, :])
        rbc = pool("rbc", 2).tile([64, 512], f32, tag="rbc")
        nc.gpsimd.partition_broadcast(rbc, recip)
        nc.vector.tensor_mul(
            ot_half[64 * h : 64 * h + 64, crel, :], ot_raw[0:64, :], rbc
        )
    # stage to the DRAM bounce in destination-major order: chunk c's four
    # 128-col stripes go to A2A ranks 4*(c%2)+s for s in 0..3 — rank d owns
    # output rows (stripe d%4, chunks == d//4 mod 2) of BOTH batches
    q = c % 2
    lj = crel // 2
    nc.sync.dma_start(
        bounce_in_part.rearrange("(d p) (j c) -> p d j c", p=128, c=128)[
            :, 4 * q : 4 * q + 4, lj, :
        ],
        ot_half.rearrange("p j (s c) -> p s j c", c=128)[:, :, crel, :],
    )


def _build_body(nc, tc, Q, K, V, Wq, Wk, Wv, Wo, Y):
    ctx_pools = {}

    def pool(name, bufs, space="SBUF"):
        if name not in ctx_pools:
            ctx_pools[name] = tc.alloc_tile_pool(name=name, bufs=bufs, space=space)
        return ctx_pools[name]

    def psum_pool(name, bufs):
        return pool(name, bufs, space="PSUM")

    const = pool("const", 1)
    ident16 = const.tile([128, 128], bf16, tag="id16")
    make_identity(nc, ident16)
    # multiplicative causal mask for a 128x128 diagonal block: keep (1)
    # where col >= row, else 0 — applied to the exp'd weights
    mask128 = const.tile([128, 128], bf16, tag="mask")
    nc.vector.memset(mask128, 1.0)
    nc.gpsimd.affine_select(
        out=mask128,
        in_=mask128,
        compare_op=ALU.is_ge,
        fill=0.0,
        base=0,
        channel_multiplier=-1,
        pattern=[[1, 128]],
    )


    mm = psum_pool("mm", 2)  # [128, 512] single-bank tiles, double-buffered
    sA = psum_pool("sA", 1)  # [128, 2, 512] scores group (even)
    sB = psum_pool("sB", 1)  # [128, 2, 512] scores group (odd)
    otp = psum_pool("otp", 2)  # [128, 512] oT accumulator

    qt_sl, kt_sl, v_sl = [], [], []
    GI = [0]

    # attention staging: heads paired per scores group (PE row-group
    # concurrency); oT written bf16, exchanged in three overlapped pieces
    # (chunks 0-3 / 4-5 / 6-8) so only the last small piece sits on the tail.
    AG_SPLIT = [(0, 4), (4, 6), (6, 8)]  # [c0, c1) chunk ranges
    # Head mixing is one 8-way AllToAll per piece, not per-batch AllGathers:
    # each core only needs the column stripes of the other heads that cover
    # ITS output rows, so routing just those moves 1/4 the bytes (collective
    # time is fixed-cost + bytes/bw; 4-core AllToAll is unsupported, hence
    # 8-way with output rows from BOTH batches per core). A2A rank d owns,
    # in each batch, the 128-row blocks 512*j + 128*(d%4) for chunks
    # j == d//4 (mod 2) — every piece carries data for every core, the
    # out-proj streams behind each piece, and all APs are rank-independent.
    ot_all = [
        pool(f"otall{i}", 1).tile(
            [128, 8, (c1 - c0) // 2, 128], bf16, tag=f"otall{i}", name=f"otall{i}"
        )
        for i, (c0, c1) in enumerate(AG_SPLIT)
    ]
    ot_parts = [
        pool("ot", 1).tile([128, c1 - c0, 512], bf16, tag=f"otp{i}", name=f"otsb{i}")
        for i, (c0, c1) in enumerate(AG_SPLIT)
    ]
    dram = pool("dram", 1, space="DRAM")
    bounce_in = [
        dram.tile([1024, (c1 - c0) * 64], bf16, tag=f"bin{i}", name=f"bin{i}")
        for i, (c0, c1) in enumerate(AG_SPLIT)
    ]
    bounce_out = [
        dram.tile([1024, (c1 - c0) * 64], bf16, tag=f"bout{i}", name=f"bout{i}")
        for i, (c0, c1) in enumerate(AG_SPLIT)
    ]

    # ---------------- phase 1: load, transpose, project ----------------
    Qr = Q.rearrange("(s g p) d -> s p g d", p=128, g=4)
    Kr = K.rearrange("(s g p) d -> s p g d", p=128, g=4)
    Vr = V.rearrange("(s g p) d -> s p g d", p=128, g=4)

    wq_sb = wk_sb = wv_sb = wo_sb = None
    for s in range(NSL):
        # --- Q and K: bf16 path (casting DMAs; bf16 transposes run 2x the
        # fp32 PE rate and their PSUM evacuations get the 2x DVE mode; fp32
        # PSUM accumulation in every matmul keeps rel err ~4e-3) ---
        xq = pool("xin", 4).tile([128, 4, 512], bf16, tag="xin")
        nc.gpsimd.dma_start(xq, Qr[s])  # casting DMA f32 -> bf16
        xk = pool("xin", 4).tile([128, 4, 512], bf16, tag="xin")
        nc.gpsimd.dma_start(xk, Kr[s])
        if s == 0:
            # weight loads queued behind the first slice so they don't
            # delay the first transposes
            wq_sb = const.tile([128, 4, 128], bf16, tag="wq")
            nc.gpsimd.dma_start(wq_sb, Wq.rearrange("(c p) k -> p c k", p=128))
            wk_sb = const.tile([128, 4, 128], bf16, tag="wk")
            nc.gpsimd.dma_start(wk_sb, Wk.rearrange("(c p) k -> p c k", p=128))
            wv_sb = const.tile([128, 4, 128], bf16, tag="wv")
            nc.gpsimd.dma_start(wv_sb, Wv.rearrange("(c p) k -> p c k", p=128))
            wo_sb = const.tile([128, 4, 512], bf16, tag="wo")
            nc.gpsimd.dma_start(wo_sb, Wo.rearrange("(c p) n -> p c n", p=128))
        xtq = pool("xt", 3).tile([128, 4, 512], bf16, tag="xtqk")
        xtk = pool("xt", 3).tile([128, 4, 512], bf16, tag="xtqk")
        for x_sl, xt_sb in ((xq, xtq), (xk, xtk)):
            for dc in range(4):
                t_ps = mm.tile([128, 512], bf16, tag="mm", name=f"tps_{s}_{dc}")
                for g in range(4):
                    nc.tensor.transpose(
                        t_ps[:, ts(g, 128)], x_sl[:, g, ts(dc, 128)], ident16
                    )
                nc.vector.tensor_copy(xt_sb[:, dc, :], t_ps)
        # qT/kT projections (both heads of the pair): [128, 512]
        qt_ps = mm.tile([128, 512], f32, tag="mm")
        for dc in range(4):
            nc.tensor.matmul(
                qt_ps,
                lhsT=wq_sb[:, dc, :],
                rhs=xtq[:, dc, :],
                start=(dc == 0),
                stop=(dc == 3),
            )
        qt = pool("qt", NSL).tile([128, 512], bf16, tag="qt")
        nc.vector.tensor_copy(qt, qt_ps)
        qt_sl.append(qt)
        kt_ps = mm.tile([128, 512], f32, tag="mm")
        for dc in range(4):
            nc.tensor.matmul(
                kt_ps,
                lhsT=wk_sb[:, dc, :],
                rhs=xtk[:, dc, :],
                start=(dc == 0),
                stop=(dc == 3),
            )
        kt = pool("kt", NSL).tile([128, 512], bf16, tag="kt")
        nc.vector.tensor_copy(kt, kt_ps)
        kt_sl.append(kt)

        # --- V: bf16 path ---
        xv = pool("xinv", 2).tile([128, 4, 512], bf16, tag="xinv")
        nc.gpsimd.dma_start(xv, Vr[s])  # casting DMA f32 -> bf16
        xtv = pool("xtv", 3).tile([128, 4, 512], bf16, tag="xtv")
        for dc in range(4):
            t_ps = mm.tile([128, 512], bf16, tag="mm", name=f"tpsv_{s}_{dc}")
            for g in range(4):
                nc.tensor.transpose(
                    t_ps[:, ts(g, 128)], xv[:, g, ts(dc, 128)], ident16
                )
            nc.vector.tensor_copy(xtv[:, dc, :], t_ps)
        # v projection, t-major: per t-chunk [128, 2*64]; interleave into
        # v_aug [128, 4, 130] with a ones column per head at 65h+64
        vp = mm.tile([128, 512], f32, tag="mm")
        for tcl in range(4):
            for dc in range(4):
                nc.tensor.matmul(
                    vp[:, ts(tcl, 128)],
                    lhsT=xtv[:, dc, ts(tcl, 128)],
                    rhs=wv_sb[:, dc, :],
                    start=(dc == 0),
                    stop=(dc == 3),
                )
        va = pool("v", NSL).tile([128, 4, 130], bf16, tag="v")
        nc.vector.memset(va.rearrange("p c (h k) -> p c h k", k=65)[:, :, :, 64:65], 1.0)
        nc.vector.tensor_copy(
            va.rearrange("p c (h k) -> p c h k", k=65)[:, :, :, 0:64],
            vp.rearrange("p (c h k) -> p c h k", c=4, h=2),
        )
        v_sl.append(va)

        part = next(i for i, (c0, c1) in enumerate(AG_SPLIT) if c0 <= s < c1)
        attention_chunk(nc, pool, sA, sB, otp, mask128, qt_sl, kt_sl, v_sl,
                        ot_parts[part], bounce_in[part],
                        s - AG_SPLIT[part][0], GI[0])
        GI[0] += 4 * (s + 1)
        if s == AG_SPLIT[part][1] - 1:
            nc.gpsimd.collective_compute(
                "AllToAll",
                ALU.bypass,
                replica_groups=[[0, 1, 2, 3, 4, 5, 6, 7]],
                ins=[bounce_in[part].opt()],
                outs=[bounce_out[part].opt()],
            )

    # ------------- phase 3: consume gathers + output projection ---------
    # All of phase 3 is emitted under a scheduling wait-hint that places it
    # after the attention stream in each in-order engine queue: the Tile
    # scheduler's collective-cost model is optimistic, and letting it hoist
    # out-proj matmuls between attention instructions makes the real PE
    # queue stall on the AllGather semaphore (measured +28us in sim). PE
    # idles for the whole tail AllGather anyway — plenty for all 8 blocks.
    # Consume DMAs sit here, not in the loop, so mid-loop bounce-in DMAs on
    # the sync queue are never stuck behind a consume's collective wait.
    def consume(part):
        # the AllToAll output is source-major: rows [128*s, 128*s+128) hold
        # core s's 128 head-features over my column stripes, contiguous —
        # sources 0-3 are batch-0 heads, 4-7 batch-1 (the tail piece is
        # spread across the three DMA-capable queues, all idle by then)
        engs = (
            (nc.sync,) * 8
            if part < 2
            else (nc.sync, nc.scalar, nc.gpsimd) * 3
        )
        for s in range(8):
            engs[s].dma_start(ot_all[part][:, s], bounce_out[part][ts(s, 128), :])

    def out_proj(bb, k):
        # my k-th owned block of batch bb: global rows 512*(par+2k) +
        # 128*(rank%4), with par = rank//4 — piece and local-chunk indices
        # are rank-independent (k<2 -> piece 0 local k; k=2/3 -> piece
        # 1/2 local 0); batch bb's head-features are A2A sources 4*bb+dc
        part, j = (0, k) if k < 2 else (k - 1, 0)
        st = 4 * bb + k
        ym = mm.tile([128, 512], f32, tag="mm", name=f"ym_{st}")
        for dc in range(4):
            nc.tensor.matmul(
                ym,
                lhsT=ot_all[part][:, 4 * bb + dc, j, :],
                rhs=wo_sb[:, dc, :],
                start=(dc == 0),
                stop=(dc == 3),
            )
        y_sb = pool("y", 4).tile([128, 512], f32, tag="y")
        if st % 2 == 0:
            nc.scalar.copy(y_sb, ym)
            nc.sync.dma_start(Y[ts(st, 128), :], y_sb)
        else:
            nc.vector.tensor_copy(y_sb, ym)
            nc.scalar.dma_start(Y[ts(st, 128), :], y_sb)

    with tc.tile_wait_until(0.20):
        consume(0)
        consume(1)
        for k in range(3):
            for bb in range(2):
                out_proj(bb, k)
    with tc.tile_wait_until(0.24):
        consume(2)
        for bb in range(2):
            out_proj(bb, 3)

    for p in reversed(list(ctx_pools.values())):
        p.release()


def _build(loop=1):
    global _CACHED_NC
    if loop == 1 and _CACHED_NC is not None:
        return _CACHED_NC
    nc = bacc.Bacc("TRN2", num_devices=8)
    Q = nc.dram_tensor("Q", [S, D], f32, kind="ExternalInput")
    K = nc.dram_tensor("K", [S, D], f32, kind="ExternalInput")
    V = nc.dram_tensor("V", [S, D], f32, kind="ExternalInput")
    Wq = nc.dram_tensor("Wq", [D, 128], f32, kind="ExternalInput")
    Wk = nc.dram_tensor("Wk", [D, 128], f32, kind="ExternalInput")
    Wv = nc.dram_tensor("Wv", [D, 128], f32, kind="ExternalInput")
    Wo = nc.dram_tensor("Wo", [D, D], f32, kind="ExternalInput")
    Y = nc.dram_tensor("Y", [1024, D], f32, kind="ExternalOutput")
    with tile.TileContext(nc) as tcx:
        for _ in range(loop):
            _build_body(nc, tcx, Q, K, V, Wq, Wk, Wv, Wo, Y)
    nc.finalize()
    if loop == 1:
        _CACHED_NC = nc
    return nc


def _in_maps(inputs):
    Q, K, V = (np.asarray(inputs[k], np.float32) for k in ("Q", "K", "V"))
    Wq, Wk, Wv, Wo = (
        np.asarray(inputs[k], np.float32) for k in ("Wq", "Wk", "Wv", "Wo")
    )
    in_maps = []
    for c in range(8):
        b, hp = c // 4, c % 4
        in_maps.append(
            {
                "Q": np.ascontiguousarray(Q[b]),
                "K": np.ascontiguousarray(K[b]),
                "V": np.ascontiguousarray(V[b]),
                "Wq": np.ascontiguousarray(
                    np.concatenate([Wq[2 * hp], Wq[2 * hp + 1]], axis=1)
                ),
                "Wk": np.ascontiguousarray(
                    np.concatenate([Wk[2 * hp], Wk[2 * hp + 1]], axis=1)
                ),
                "Wv": np.ascontiguousarray(
                    np.concatenate([Wv[2 * hp], Wv[2 * hp + 1]], axis=1)
                ),
                "Wo": Wo,
            }
        )
    return in_maps


def _assemble(per_core_results):
    # A2A rank d returns Y blocks st = 4*bb + k: batch bb's global rows
    # 512*(d//4 + 2*k) + 128*(d%4) .. +128
    out = np.empty((B, S, D), np.float32)
    for d in range(8):
        par, sp = d // 4, d % 4
        y = per_core_results[d]["Y"].reshape(2, 4, 128, D)
        for bb in range(2):
            for k in range(4):
                r0 = 512 * (par + 2 * k) + 128 * sp
                out[bb, r0 : r0 + 128] = y[bb, k]
    return out


def kernel(Q, K, V, Wq, Wk, Wv, Wo):
    nc = _build()
    in_maps = _in_maps(
        {"Q": Q, "K": K, "V": V, "Wq": Wq, "Wk": Wk, "Wv": Wv, "Wo": Wo}
    )
    res = run_bass_kernel_spmd(nc, in_maps, core_ids=list(range(8)))
    return _assemble(res.results)

